# revision 18
# baseline (speedup 1.0000x reference)
"""Trainium2 Bass kernel for nn_BertContrastivePredictor.

Sharding: data-parallel over batch, 4 samples per core (8 cores).
Per core: 64 spans; fwd lanes on partitions 0-63, bwd on 64-127.

Key structure per core:
  - hid is shipped to DRAM in bf16; gpsimd dma_gather(transpose=True)
    lands span tokens directly in x^T layout (no PE transposes) and a
    second sequential-index gather produces hid^T for the attention.
  - LSTM recurrence: per step one [128, 2048] PSUM group accumulates
    Wih x_t (both dirs) + Whh h_{t-1}; gates ordered (i,f,o,g) so one
    fused Sigmoid covers i,f,o straight from PSUM; tanh for g; DVE does
    the cell update; h^T via 4 PE transposes into a consumed PSUM bank.
  - Attention: block-diagonal over samples with a host-built mask:
    attn = sfT^T @ hidT (M=64), masked to bf16, DMA-xbar transposed,
    ctx = attnT^T @ hid_nat accumulated over row chunks.

labels sim-part is computed on host: the reference divides by sim
row-sums as small as 7e-5 (outputs up to ~600), which reduced-precision
device math cannot reproduce; it is ~0.5% of model FLOPs.

Output per core [64, 2080] = [slot_feats(1024) | context(1024) | labels(32)].
"""

import contextlib

import numpy as np
import ml_dtypes

import concourse.bass as bass
import concourse.bacc as bacc
import concourse.tile as tile
import concourse.mybir as mybir
from concourse import bass_utils
from concourse import library_config

f32 = mybir.dt.float32
bf16 = mybir.dt.bfloat16
i16 = mybir.dt.int16
AF = mybir.ActivationFunctionType
OP = mybir.AluOpType

B, S, D, H, K, L, NS, NT = 32, 512, 1024, 512, 16, 16, 16, 16
SMOOTH = 0.1
EPS = 1e-8
NCORES = 8
BL = B // NCORES            # local batch = 4
NSP = BL * K                # local spans = 64
G4 = 4 * H                  # 2048 gates per direction
OUTW = 2 * H + D + NS + NT  # 2080
ROWS = BL * S               # 2048 hidden rows per core
PERM = (2, 0, 1, 3)         # torch gate order (i,f,g,o) -> (g,i,f,o)


def build_program(dbg=False, reps=1, has_bias=False):
    nc = bacc.Bacc("TRN2", target_bir_lowering=False, debug=False,
                   num_swdge_queues=4)

    hid_d = nc.dram_tensor("hid", [ROWS, D], bf16, kind="ExternalInput")
    gx_d = nc.dram_tensor("gx", [4, 128, 32], i16, kind="ExternalInput")
    gh_d = nc.dram_tensor("gh", [4, 128, 32], i16, kind="ExternalInput")
    wih_d = nc.dram_tensor("wih", [128, 2, 8, G4], bf16, kind="ExternalInput")
    whh_d = nc.dram_tensor("whh", [128, 2, 4, G4], bf16, kind="ExternalInput")
    cmask_d = nc.dram_tensor("cmask", [NSP, BL, S], bf16, kind="ExternalInput")
    labs_d = nc.dram_tensor("labs", [NSP, NS + NT], f32, kind="ExternalInput")
    idb_d = nc.dram_tensor("idb", [128, 128], bf16, kind="ExternalInput")
    bias2_d = nc.dram_tensor("bias2", [2, G4], bf16, kind="ExternalInput")
    ind2_d = nc.dram_tensor("ind2", [2, 128], bf16, kind="ExternalInput")
    out_d = nc.dram_tensor("out", [NSP, OUTW], f32, kind="ExternalOutput")

    with tile.TileContext(nc, pool_alloc_mode="queue") as tc:
        with tc.tile_pool(name="cst", bufs=1) as cst:
            cs = {
                "cmask": cst.tile([NSP, BL * S], bf16, name="cmask"),
                "sf_acc": cst.tile([128, H], f32, name="sf_acc"),
                "gxi": cst.tile([128, 4, 32], i16, name="gxi"),
                "ghi": cst.tile([128, 4, 32], i16, name="ghi"),
            }
            if has_bias:
                cs["bias2"] = cst.tile([2, G4], bf16, name="bias2")
                cs["ind2"] = cst.tile([2, 128], bf16, name="ind2")
            for _ in range(reps):
                _build(nc, tc, cs, hid_d, gx_d, gh_d, wih_d, whh_d, cmask_d,
                       labs_d, idb_d, bias2_d, ind2_d, out_d, has_bias)
    nc.compile()
    return nc


def _build(nc, tc, cs, hid_d, gx_d, gh_d, wih_d, whh_d, cmask_d,
           labs_d, idb_d, bias2_d, ind2_d, out_d, has_bias):
    est = contextlib.ExitStack()
    MM = nc.tensor.matmul

    nc.gpsimd.load_library(library_config.mlp)

    # ---------- constants / persistent ----------
    cmask, sf_acc = cs["cmask"], cs["sf_acc"]
    gxi, ghi = cs["gxi"], cs["ghi"]
    nc.sync.dma_start(cmask[:], cmask_d.ap())
    nc.sync.dma_start(gxi[:], gx_d.ap().rearrange("g p s -> p g s"))
    nc.sync.dma_start(ghi[:], gh_d.ap().rearrange("g p s -> p g s"))
    if has_bias:
        bias2, ind2 = cs["bias2"], cs["ind2"]
        nc.sync.dma_start(bias2[:], bias2_d.ap())
        nc.sync.dma_start(ind2[:], ind2_d.ap())

    nc.sync.dma_start(out_d.ap()[:, 2 * H + D:], labs_d.ap())

    # ---------- weights ----------
    wts = est.enter_context(tc.tile_pool(name="wts", bufs=1))
    wih = wts.tile([128, 2, 8, G4], bf16)
    for c in range(8):
        nc.sync.dma_start(wih[:, :, c, :], wih_d.ap()[:, :, c, :])
    whh = wts.tile([128, 2, 4, G4], bf16)
    nc.sync.dma_start(whh[:], whh_d.ap())

    # ---------- gathers: xT (span tokens) and hidT, both pre-transposed ----
    hap = hid_d.ap()
    in_ap = bass.AP(tensor=hap.tensor, offset=0, ap=[[D, ROWS], [1, D]])
    xt_pool = est.enter_context(tc.tile_pool(name="xt", bufs=1))
    xtq = []
    for q in range(4):
        t = xt_pool.tile([128, 8, 512], bf16, name=f"xtq{q}")
        nc.gpsimd.dma_gather(
            out_ap=t[:], in_ap=in_ap, idxs_ap=gxi[:, q, :],
            num_idxs=512, num_idxs_reg=512, elem_size=D, elem_step=D,
            transpose=True, queue_num=q)
        xtq.append(t)
    ht_pool = est.enter_context(tc.tile_pool(name="ht", bufs=1))
    hidq = []
    for q in range(4):
        t = ht_pool.tile([128, 8, 512], bf16, name=f"hidq{q}")
        nc.gpsimd.dma_gather(
            out_ap=t[:], in_ap=in_ap, idxs_ap=ghi[:, q, :],
            num_idxs=512, num_idxs_reg=512, elem_size=D, elem_step=D,
            transpose=True, queue_num=q)
        hidq.append(t)

    # ---------- LSTM recurrence ----------
    with tc.tile_pool(name="rec", bufs=2) as rec, \
         tc.tile_pool(name="mps", bufs=2, space="PSUM") as mps:
        hT_prev = None
        c_prev = None
        for tau in range(L):
            xq, xo = divmod(tau, 4)
            ps = mps.tile([128, G4], f32, tag="ps", name=f"ps{tau}")
            if has_bias:
                MM(ps[:], ind2[:], bias2[:], start=True, stop=False,
                   skip_group_check=True)
            # gate-outer so consecutive matmuls never share a stationary
            # operand (same-lhsT runs measure ~2.5x slower per matmul)
            for g in range(4):
                gs = slice(g * 512, (g + 1) * 512)
                for c8 in range(8):
                    lhf = xtq[xq][:, c8, xo * 128: xo * 128 + 64]
                    lhb = xtq[xq][:, c8, xo * 128 + 64: xo * 128 + 128]
                    st0 = (c8 == 0) and not has_bias
                    stp = (tau == 0) and (c8 == 7)
                    MM(ps[0:64, gs], lhf, wih[:, 0, c8, gs],
                       start=st0, stop=stp, skip_group_check=True)
                    MM(ps[64:128, gs], lhb, wih[:, 1, c8, gs],
                       start=st0, stop=stp, skip_group_check=True)
            if tau > 0:
                # bank-major, gate order (g,i,f,o): g's bank stops first so
                # its tanh overlaps the remaining Whh banks
                for g in range(4):
                    gs = slice(g * 512, (g + 1) * 512)
                    for hc in range(4):
                        stp = hc == 3
                        MM(ps[0:64, gs], hT_prev[:, hc, 0:64],
                           whh[:, 0, hc, gs], start=False, stop=stp,
                           skip_group_check=True)
                        MM(ps[64:128, gs], hT_prev[:, hc, 64:128],
                           whh[:, 1, hc, gs], start=False, stop=stp,
                           skip_group_check=True)

            gtan = rec.tile([128, 512], bf16, tag="gtan")
            nc.scalar.activation(gtan[:], ps[:, 0:512], AF.Tanh)
            gsig = rec.tile([128, 3 * 512], bf16, tag="gsig")
            nc.scalar.activation(gsig[:, 0:512], ps[:, 512:1024], AF.Sigmoid)
            nc.scalar.activation(gsig[:, 512:1536], ps[:, 1024:2048],
                                 AF.Sigmoid)
            ig = rec.tile([128, 512], bf16, tag="ig")
            nc.vector.tensor_tensor(out=ig[:], in0=gsig[:, 0:512],
                                    in1=gtan[:], op=OP.mult)
            c_new = rec.tile([128, 512], f32, tag="c")
            if tau == 0:
                nc.vector.tensor_copy(out=c_new[:], in_=ig[:])
            else:
                fc = rec.tile([128, 512], f32, tag="fc")
                nc.vector.tensor_tensor(out=fc[:], in0=gsig[:, 512:1024],
                                        in1=c_prev[:], op=OP.mult)
                nc.vector.tensor_tensor(out=c_new[:], in0=ig[:],
                                        in1=fc[:], op=OP.add)
            th = rec.tile([128, 512], bf16, tag="th")
            nc.scalar.activation(th[:], c_new[:], AF.Tanh)
            h_new = rec.tile([128, 512], bf16, tag="h")
            nc.vector.tensor_tensor(out=h_new[:], in0=gsig[:, 1024:1536],
                                    in1=th[:], op=OP.mult)
            if tau == 0:
                nc.vector.tensor_copy(out=sf_acc[:], in_=h_new[:])
            else:
                nc.vector.tensor_tensor(out=sf_acc[:], in0=sf_acc[:],
                                        in1=h_new[:], op=OP.add)
            if tau < L - 1:
                hT = rec.tile([128, 4, 128], bf16, tag="hT")
                nc.sync.dma_start_transpose(hT[:], h_new[:])
                hT_prev = hT
            c_prev = c_new

    # slot_feats out
    nc.sync.dma_start(out_d.ap()[:, 0:H], sf_acc[0:64, :])
    nc.sync.dma_start(out_d.ap()[:, H:2 * H], sf_acc[64:128, :])

    # ---------- attention ----------
    with tc.tile_pool(name="asb", bufs=1) as asb, \
         tc.tile_pool(name="aps", bufs=1, space="PSUM") as aps:
        sf_bf = asb.tile([128, H], bf16)
        nc.vector.tensor_copy(out=sf_bf[:], in_=sf_acc[:])
        sfT_f = asb.tile([128, 4, 64], bf16)
        nc.sync.dma_start_transpose(sfT_f[:], sf_bf[0:64, :])
        sfT_b = asb.tile([128, 4, 64], bf16)
        nc.sync.dma_start_transpose(sfT_b[:], sf_bf[64:128, :])

        # pipelined by sample-quarter: attn(nq) -> mask -> transpose ->
        # ctx MMs for that quarter run while quarter nq+1 accumulates
        ps_at = aps.tile([NSP, BL * S], f32)
        at2 = asb.tile([NSP, BL * S], bf16)
        atT = asb.tile([128, 16, 64], bf16)
        ps_ctx = aps.tile([NSP, D], f32)
        ctx_sb = asb.tile([NSP, D], f32)
        with tc.tile_pool(name="hnat", bufs=8) as hnat:
            for nq in range(BL):
                qs = slice(nq * 512, (nq + 1) * 512)
                for dc in range(8):
                    lh = (sfT_f if dc < 4 else sfT_b)[:, dc % 4, :]
                    MM(ps_at[:, qs], lh, hidq[nq][:, dc, :],
                       start=(dc == 0), stop=(dc == 7),
                       skip_group_check=True)
                nc.vector.tensor_tensor(out=at2[:, qs], in0=ps_at[:, qs],
                                        in1=cmask[:, qs], op=OP.mult)
                nc.sync.dma_start_transpose(
                    atT[:, 4 * nq: 4 * nq + 4, :], at2[:, qs])
                for si in range(4):
                    sc = 4 * nq + si
                    hn = hnat.tile([128, D], bf16, tag="hn", name=f"hn{sc}")
                    nc.sync.dma_start(hn[:], hap[sc * 128:(sc + 1) * 128, :])
                    for f2 in range(2):
                        MM(ps_ctx[:, f2 * 512:(f2 + 1) * 512], atT[:, sc, :],
                           hn[:, f2 * 512:(f2 + 1) * 512],
                           start=(sc == 0), stop=(sc == 15),
                           skip_group_check=True)
        nc.vector.tensor_copy(out=ctx_sb[:], in_=ps_ctx[:])
        nc.sync.dma_start(out_d.ap()[:, 2 * H: 2 * H + D], ctx_sb[:])

    est.close()


# ---------------- host side ----------------

def _mlp_np(x, W1, b1, W2, b2):
    return np.tanh(x @ W1.T + b1) @ W2.T + b2


def _wrap_idx(idx512):
    g = np.zeros((16, 32), np.int16)
    for i in range(512):
        g[i % 16, i // 16] = idx512[i]
    return np.tile(g, (8, 1))


def prep_core_inputs(inp, ci):
    b0 = ci * BL
    hid = np.asarray(inp["hidden_layers"][b0:b0 + BL],
                     np.float32).reshape(ROWS, D)
    hid_bf = np.ascontiguousarray(hid.astype(ml_dtypes.bfloat16))

    span_idx = np.asarray(inp["span_idx"][b0:b0 + BL], np.int64)  # [BL,K,L]
    gx = np.zeros((4, 128, 32), np.int16)
    for tq in range(4):
        idxs = np.zeros(512, np.int64)
        for i in range(512):
            tau = 4 * tq + i // 128
            lane = i % 128
            b, k = divmod(lane % 64, K)
            t = tau if lane < 64 else (L - 1 - tau)
            idxs[i] = b * S + span_idx[b, k, t]
        gx[tq] = _wrap_idx(idxs)
    gh = np.zeros((4, 128, 32), np.int16)
    for tq in range(4):
        gh[tq] = _wrap_idx(np.arange(tq * 512, (tq + 1) * 512))

    def wr(w):  # [4H, Din] -> [128, Din/128, 4H] with gate perm
        wt = np.asarray(w, np.float32)
        din = wt.shape[1]
        wt = wt.reshape(4, H, din)[list(PERM)].reshape(G4, din).T
        cn = din // 128
        return wt.reshape(cn, 128, G4).transpose(1, 0, 2)

    wih = np.stack([wr(inp["Wih_f"]), wr(inp["Wih_b"])], axis=1)
    whh = np.stack([wr(inp["Whh_f"]), wr(inp["Whh_b"])], axis=1)

    def bperm(bih, bhh):
        v = (np.asarray(bih, np.float32) + np.asarray(bhh, np.float32))
        return v.reshape(4, H)[list(PERM)].reshape(G4)

    bias2 = np.stack([bperm(inp["bih_f"], inp["bhh_f"]),
                      bperm(inp["bih_b"], inp["bhh_b"])])
    has_bias = bool(np.any(bias2 != 0.0))
    ind2 = np.zeros((2, 128), np.float32)
    ind2[0, 0:64] = 1.0
    ind2[1, 64:128] = 1.0

    # context mask, block-diagonal over samples: [64, BL, S]
    ss = np.asarray(inp["span_start"][b0:b0 + BL], np.int64)
    se = np.asarray(inp["span_end"][b0:b0 + BL], np.int64)
    ln = np.asarray(inp["length"][b0:b0 + BL], np.int64)
    pos = np.arange(S)
    cmask = np.zeros((BL, K, BL, S), np.float32)
    for b in range(BL):
        m = ((pos[None, :] < ss[b][:, None])
             | ((pos[None, :] > se[b][:, None])
                & (pos[None, :] < ln[b])))
        cmask[b, :, b, :] = m
    cmask = cmask.reshape(NSP, BL, S)

    # labels on host (fp32): one-hot*SMOOTH | sim normalized
    se_ = np.asarray(inp["slot_emb"][b0:b0 + BL], np.float32).reshape(NSP, D)
    tgt = np.asarray(inp["tgt_slot_embs"], np.float32)

    def mlp32(x, w1, bb1, w2, bb2):
        return _mlp_np(x, np.asarray(w1, np.float32),
                       np.asarray(bb1, np.float32),
                       np.asarray(w2, np.float32),
                       np.asarray(bb2, np.float32))

    s_cat = np.concatenate([
        mlp32(se_, inp["Wps1"], inp["bps1"], inp["Wps2"], inp["bps2"]),
        mlp32(se_, inp["Wpc1"], inp["bpc1"], inp["Wpc2"], inp["bpc2"])],
        axis=-1)
    t_cat = np.concatenate([
        mlp32(tgt, inp["Wps1"], inp["bps1"], inp["Wps2"], inp["bps2"]),
        mlp32(tgt, inp["Wpc1"], inp["bpc1"], inp["Wpc2"], inp["bpc2"])],
        axis=-1)
    sn = np.maximum(np.linalg.norm(s_cat, axis=-1), EPS)
    tn = np.maximum(np.linalg.norm(t_cat, axis=-1), EPS)
    sim = (s_cat @ t_cat.T) / (sn[:, None] * tn[None, :])
    labsim = (sim / sim.sum(axis=-1, keepdims=True) * (1.0 - SMOOTH))
    sid = np.asarray(inp["src_slot_ids"][b0:b0 + BL], np.int64).reshape(NSP)
    oh = np.zeros((NSP, NS), np.float32)
    oh[np.arange(NSP), sid] = SMOOTH
    labs = np.concatenate([oh, labsim.astype(np.float32)], axis=1)

    def bf(a):
        return np.ascontiguousarray(np.asarray(a).astype(ml_dtypes.bfloat16))

    return {
        "hid": hid_bf, "gx": gx, "gh": gh,
        "wih": bf(wih), "whh": bf(whh),
        "cmask": bf(cmask), "labs": labs.astype(np.float32),
        "idb": bf(np.eye(128)), "bias2": bf(bias2), "ind2": bf(ind2),
    }, has_bias


_NC_CACHE = {}


def _get_nc(has_bias=False):
    if has_bias not in _NC_CACHE:
        _NC_CACHE[has_bias] = build_program(has_bias=has_bias)
    return _NC_CACHE[has_bias]


def kernel(**inputs):
    preps = [prep_core_inputs(inputs, ci) for ci in range(NCORES)]
    has_bias = any(p[1] for p in preps)
    in_maps = [p[0] for p in preps]
    nc = _get_nc(has_bias)
    res = bass_utils.run_bass_kernel_spmd(nc, in_maps, list(range(NCORES)))
    outs = [res.results[i]["out"].reshape(BL, K, OUTW) for i in range(NCORES)]
    return np.concatenate(outs, axis=0)


# revision 20
# speedup vs baseline: 1.0586x; 1.0586x over previous
"""Trainium2 Bass kernel for nn_BertContrastivePredictor.

Sharding: data-parallel over batch, 4 samples per core (8 cores).
Per core: 64 spans; fwd lanes on partitions 0-63, bwd on 64-127.

Key structure per core:
  - hid is shipped to DRAM in bf16; gpsimd dma_gather(transpose=True)
    lands span tokens directly in x^T layout (no PE transposes) and a
    second sequential-index gather produces hid^T for the attention.
  - LSTM recurrence: per step one [128, 2048] PSUM group accumulates
    Wih x_t (both dirs) + Whh h_{t-1}; gates ordered (i,f,o,g) so one
    fused Sigmoid covers i,f,o straight from PSUM; tanh for g; DVE does
    the cell update; h^T via 4 PE transposes into a consumed PSUM bank.
  - Attention: block-diagonal over samples with a host-built mask:
    attn = sfT^T @ hidT (M=64), masked to bf16, DMA-xbar transposed,
    ctx = attnT^T @ hid_nat accumulated over row chunks.

labels sim-part is computed on host: the reference divides by sim
row-sums as small as 7e-5 (outputs up to ~600), which reduced-precision
device math cannot reproduce; it is ~0.5% of model FLOPs.

Output per core [64, 2080] = [slot_feats(1024) | context(1024) | labels(32)].
"""

import contextlib

import numpy as np
import ml_dtypes

import concourse.bass as bass
import concourse.bacc as bacc
import concourse.tile as tile
import concourse.mybir as mybir
from concourse import bass_utils
from concourse import library_config

f32 = mybir.dt.float32
bf16 = mybir.dt.bfloat16
i16 = mybir.dt.int16
AF = mybir.ActivationFunctionType
OP = mybir.AluOpType

B, S, D, H, K, L, NS, NT = 32, 512, 1024, 512, 16, 16, 16, 16
SMOOTH = 0.1
EPS = 1e-8
NCORES = 8
BL = B // NCORES            # local batch = 4
NSP = BL * K                # local spans = 64
G4 = 4 * H                  # 2048 gates per direction
OUTW = 2 * H + D + NS + NT  # 2080
ROWS = BL * S               # 2048 hidden rows per core
PERM = (2, 0, 1, 3)         # torch gate order (i,f,g,o) -> (g,i,f,o)


def build_program(dbg=False, reps=1, has_bias=False):
    nc = bacc.Bacc("TRN2", target_bir_lowering=False, debug=False,
                   num_swdge_queues=4)

    hid_d = nc.dram_tensor("hid", [ROWS, D], bf16, kind="ExternalInput")
    gx_d = nc.dram_tensor("gx", [4, 128, 32], i16, kind="ExternalInput")
    gh_d = nc.dram_tensor("gh", [4, 128, 32], i16, kind="ExternalInput")
    wih_d = nc.dram_tensor("wih", [128, 2, 8, G4], bf16, kind="ExternalInput")
    whh_d = nc.dram_tensor("whh", [128, 2, 4, G4], bf16, kind="ExternalInput")
    cmask_d = nc.dram_tensor("cmask", [NSP, BL, S], bf16, kind="ExternalInput")
    labs_d = nc.dram_tensor("labs", [NSP, NS + NT], f32, kind="ExternalInput")
    idb_d = nc.dram_tensor("idb", [128, 128], bf16, kind="ExternalInput")
    bias2_d = nc.dram_tensor("bias2", [2, G4], bf16, kind="ExternalInput")
    ind2_d = nc.dram_tensor("ind2", [2, 128], bf16, kind="ExternalInput")
    out_d = nc.dram_tensor("out", [NSP, OUTW], f32, kind="ExternalOutput")

    with tile.TileContext(nc, pool_alloc_mode="queue") as tc:
        with tc.tile_pool(name="cst", bufs=1) as cst:
            cs = {
                "cmask": cst.tile([NSP, BL * S], bf16, name="cmask"),
                "sf_acc": cst.tile([128, H], f32, name="sf_acc"),
                "gxi": cst.tile([128, 4, 32], i16, name="gxi"),
                "ghi": cst.tile([128, 4, 32], i16, name="ghi"),
            }
            if has_bias:
                cs["bias2"] = cst.tile([2, G4], bf16, name="bias2")
                cs["ind2"] = cst.tile([2, 128], bf16, name="ind2")
            for _ in range(reps):
                _build(nc, tc, cs, hid_d, gx_d, gh_d, wih_d, whh_d, cmask_d,
                       labs_d, idb_d, bias2_d, ind2_d, out_d, has_bias)
    nc.compile()
    return nc


def _build(nc, tc, cs, hid_d, gx_d, gh_d, wih_d, whh_d, cmask_d,
           labs_d, idb_d, bias2_d, ind2_d, out_d, has_bias):
    est = contextlib.ExitStack()
    MM = nc.tensor.matmul

    nc.gpsimd.load_library(library_config.mlp)

    # ---------- constants / persistent ----------
    cmask, sf_acc = cs["cmask"], cs["sf_acc"]
    gxi, ghi = cs["gxi"], cs["ghi"]
    nc.sync.dma_start(cmask[:], cmask_d.ap())
    nc.sync.dma_start(gxi[:], gx_d.ap().rearrange("g p s -> p g s"))
    nc.sync.dma_start(ghi[:], gh_d.ap().rearrange("g p s -> p g s"))
    if has_bias:
        bias2, ind2 = cs["bias2"], cs["ind2"]
        nc.sync.dma_start(bias2[:], bias2_d.ap())
        nc.sync.dma_start(ind2[:], ind2_d.ap())

    nc.sync.dma_start(out_d.ap()[:, 2 * H + D:], labs_d.ap())

    # ---------- weights ----------
    wts = est.enter_context(tc.tile_pool(name="wts", bufs=1))
    wih = wts.tile([128, 2, 8, G4], bf16)
    for c in range(8):
        nc.sync.dma_start(wih[:, :, c, :], wih_d.ap()[:, :, c, :])
    whh = wts.tile([128, 2, 4, G4], bf16)
    nc.sync.dma_start(whh[:], whh_d.ap())

    # ---------- gathers: xT (span tokens) and hidT, both pre-transposed ----
    hap = hid_d.ap()
    in_ap = bass.AP(tensor=hap.tensor, offset=0, ap=[[D, ROWS], [1, D]])
    xt_pool = est.enter_context(tc.tile_pool(name="xt", bufs=1))
    xtq = []
    for q in range(4):
        t = xt_pool.tile([128, 8, 512], bf16, name=f"xtq{q}")
        nc.gpsimd.dma_gather(
            out_ap=t[:], in_ap=in_ap, idxs_ap=gxi[:, q, :],
            num_idxs=512, num_idxs_reg=512, elem_size=D, elem_step=D,
            transpose=True, queue_num=q)
        xtq.append(t)
    ht_pool = est.enter_context(tc.tile_pool(name="ht", bufs=1))
    hidq = []
    for q in range(4):
        t = ht_pool.tile([128, 8, 512], bf16, name=f"hidq{q}")
        nc.gpsimd.dma_gather(
            out_ap=t[:], in_ap=in_ap, idxs_ap=ghi[:, q, :],
            num_idxs=512, num_idxs_reg=512, elem_size=D, elem_step=D,
            transpose=True, queue_num=q)
        hidq.append(t)

    # ---------- LSTM recurrence ----------
    with tc.tile_pool(name="rec", bufs=2) as rec, \
         tc.tile_pool(name="mps", bufs=2, space="PSUM") as mps:
        hT_prev = None
        c_prev = None
        for tau in range(L):
            xq, xo = divmod(tau, 4)
            ps = mps.tile([128, G4], f32, tag="ps", name=f"ps{tau}")
            if has_bias:
                MM(ps[:], ind2[:], bias2[:], start=True, stop=False,
                   skip_group_check=True)
            # gate-outer so consecutive matmuls never share a stationary
            # operand (same-lhsT runs measure ~2.5x slower per matmul)
            for g in range(4):
                gs = slice(g * 512, (g + 1) * 512)
                for c8 in range(8):
                    lhf = xtq[xq][:, c8, xo * 128: xo * 128 + 64]
                    lhb = xtq[xq][:, c8, xo * 128 + 64: xo * 128 + 128]
                    st0 = (c8 == 0) and not has_bias
                    stp = (tau == 0) and (c8 == 7)
                    MM(ps[0:64, gs], lhf, wih[:, 0, c8, gs],
                       start=st0, stop=stp, skip_group_check=True)
                    MM(ps[64:128, gs], lhb, wih[:, 1, c8, gs],
                       start=st0, stop=stp, skip_group_check=True)
            if tau > 0:
                # bank-major, gate order (g,i,f,o): g's bank stops first so
                # its tanh overlaps the remaining Whh banks
                for g in range(4):
                    gs = slice(g * 512, (g + 1) * 512)
                    for hc in range(4):
                        stp = hc == 3
                        MM(ps[0:64, gs], hT_prev[:, hc, 0:64],
                           whh[:, 0, hc, gs], start=False, stop=stp,
                           skip_group_check=True)
                        MM(ps[64:128, gs], hT_prev[:, hc, 64:128],
                           whh[:, 1, hc, gs], start=False, stop=stp,
                           skip_group_check=True)

            gtan = rec.tile([128, 512], bf16, tag="gtan")
            nc.scalar.activation(gtan[:], ps[:, 0:512], AF.Tanh)
            gsig = rec.tile([128, 3 * 512], bf16, tag="gsig")
            nc.scalar.activation(gsig[:, 0:512], ps[:, 512:1024], AF.Sigmoid)
            nc.scalar.activation(gsig[:, 512:1536], ps[:, 1024:2048],
                                 AF.Sigmoid)
            ig = rec.tile([128, 512], bf16, tag="ig")
            nc.vector.tensor_tensor(out=ig[:], in0=gsig[:, 0:512],
                                    in1=gtan[:], op=OP.mult)
            c_new = rec.tile([128, 512], f32, tag="c")
            if tau == 0:
                nc.vector.tensor_copy(out=c_new[:], in_=ig[:])
            else:
                fc = rec.tile([128, 512], f32, tag="fc")
                nc.vector.tensor_tensor(out=fc[:], in0=gsig[:, 512:1024],
                                        in1=c_prev[:], op=OP.mult)
                nc.vector.tensor_tensor(out=c_new[:], in0=ig[:],
                                        in1=fc[:], op=OP.add)
            th = rec.tile([128, 512], bf16, tag="th")
            nc.scalar.activation(th[:], c_new[:], AF.Tanh)
            h_new = rec.tile([128, 512], bf16, tag="h")
            nc.vector.tensor_tensor(out=h_new[:], in0=gsig[:, 1024:1536],
                                    in1=th[:], op=OP.mult)
            if tau == 0:
                nc.vector.tensor_copy(out=sf_acc[:], in_=h_new[:])
            else:
                nc.vector.tensor_tensor(out=sf_acc[:], in0=sf_acc[:],
                                        in1=h_new[:], op=OP.add)
            if tau < L - 1:
                hT = rec.tile([128, 4, 128], bf16, tag="hT")
                nc.sync.dma_start_transpose(hT[:], h_new[:])
                hT_prev = hT
            c_prev = c_new

    # slot_feats out
    nc.sync.dma_start(out_d.ap()[:, 0:H], sf_acc[0:64, :])
    nc.sync.dma_start(out_d.ap()[:, H:2 * H], sf_acc[64:128, :])

    # ---------- attention ----------
    with tc.tile_pool(name="asb", bufs=1) as asb, \
         tc.tile_pool(name="aps", bufs=1, space="PSUM") as aps:
        sf_bf = asb.tile([128, H], bf16)
        nc.vector.tensor_copy(out=sf_bf[:], in_=sf_acc[:])
        sfT_f = asb.tile([128, 4, 64], bf16)
        nc.sync.dma_start_transpose(sfT_f[:], sf_bf[0:64, :])
        sfT_b = asb.tile([128, 4, 64], bf16)
        nc.sync.dma_start_transpose(sfT_b[:], sf_bf[64:128, :])

        # pipelined by sample-quarter: attn(nq) -> mask -> transpose ->
        # ctx MMs for that quarter run while quarter nq+1 accumulates
        ps_at = aps.tile([NSP, BL * S], f32)
        at2 = asb.tile([NSP, BL * S], bf16)
        atT = asb.tile([128, 16, 64], bf16)
        ps_ctx = aps.tile([NSP, D], f32)
        ctx_sb = asb.tile([NSP, D], f32)
        with tc.tile_pool(name="hnat", bufs=8) as hnat:
            for nq in range(BL):
                qs = slice(nq * 512, (nq + 1) * 512)
                for dc in range(8):
                    lh = (sfT_f if dc < 4 else sfT_b)[:, dc % 4, :]
                    MM(ps_at[:, qs], lh, hidq[nq][:, dc, :],
                       start=(dc == 0), stop=(dc == 7),
                       skip_group_check=True)
                nc.vector.tensor_tensor(out=at2[:, qs], in0=ps_at[:, qs],
                                        in1=cmask[:, qs], op=OP.mult)
                nc.sync.dma_start_transpose(
                    atT[:, 4 * nq: 4 * nq + 4, :], at2[:, qs])
                for si in range(4):
                    sc = 4 * nq + si
                    hn = hnat.tile([128, D], bf16, tag="hn", name=f"hn{sc}")
                    nc.sync.dma_start(hn[:], hap[sc * 128:(sc + 1) * 128, :])
                    for f2 in range(2):
                        MM(ps_ctx[:, f2 * 512:(f2 + 1) * 512], atT[:, sc, :],
                           hn[:, f2 * 512:(f2 + 1) * 512],
                           start=(sc == 0), stop=(sc == 15),
                           skip_group_check=True)
        nc.vector.tensor_copy(out=ctx_sb[:], in_=ps_ctx[:])
        nc.sync.dma_start(out_d.ap()[:, 2 * H: 2 * H + D], ctx_sb[:])

    est.close()


# ---------------- host side ----------------

def _mlp_np(x, W1, b1, W2, b2):
    return np.tanh(x @ W1.T + b1) @ W2.T + b2


def _wrap_idx(idx512):
    g = np.zeros((16, 32), np.int16)
    for i in range(512):
        g[i % 16, i // 16] = idx512[i]
    return np.tile(g, (8, 1))


def prep_core_inputs(inp, ci):
    b0 = ci * BL
    hid = np.asarray(inp["hidden_layers"][b0:b0 + BL],
                     np.float32).reshape(ROWS, D)
    hid_bf = np.ascontiguousarray(hid.astype(ml_dtypes.bfloat16))

    span_idx = np.asarray(inp["span_idx"][b0:b0 + BL], np.int64)  # [BL,K,L]
    gx = np.zeros((4, 128, 32), np.int16)
    for tq in range(4):
        idxs = np.zeros(512, np.int64)
        for i in range(512):
            tau = 4 * tq + i // 128
            lane = i % 128
            b, k = divmod(lane % 64, K)
            t = tau if lane < 64 else (L - 1 - tau)
            idxs[i] = b * S + span_idx[b, k, t]
        gx[tq] = _wrap_idx(idxs)
    gh = np.zeros((4, 128, 32), np.int16)
    for tq in range(4):
        gh[tq] = _wrap_idx(np.arange(tq * 512, (tq + 1) * 512))

    def wr(w):  # [4H, Din] -> [128, Din/128, 4H] with gate perm
        wt = np.asarray(w, np.float32)
        din = wt.shape[1]
        wt = wt.reshape(4, H, din)[list(PERM)].reshape(G4, din).T
        cn = din // 128
        return wt.reshape(cn, 128, G4).transpose(1, 0, 2)

    wih = np.stack([wr(inp["Wih_f"]), wr(inp["Wih_b"])], axis=1)
    whh = np.stack([wr(inp["Whh_f"]), wr(inp["Whh_b"])], axis=1)

    def bperm(bih, bhh):
        v = (np.asarray(bih, np.float32) + np.asarray(bhh, np.float32))
        return v.reshape(4, H)[list(PERM)].reshape(G4)

    bias2 = np.stack([bperm(inp["bih_f"], inp["bhh_f"]),
                      bperm(inp["bih_b"], inp["bhh_b"])])
    has_bias = bool(np.any(bias2 != 0.0))
    ind2 = np.zeros((2, 128), np.float32)
    ind2[0, 0:64] = 1.0
    ind2[1, 64:128] = 1.0

    # context mask, block-diagonal over samples: [64, BL, S]
    ss = np.asarray(inp["span_start"][b0:b0 + BL], np.int64)
    se = np.asarray(inp["span_end"][b0:b0 + BL], np.int64)
    ln = np.asarray(inp["length"][b0:b0 + BL], np.int64)
    pos = np.arange(S)
    cmask = np.zeros((BL, K, BL, S), np.float32)
    for b in range(BL):
        m = ((pos[None, :] < ss[b][:, None])
             | ((pos[None, :] > se[b][:, None])
                & (pos[None, :] < ln[b])))
        cmask[b, :, b, :] = m
    cmask = cmask.reshape(NSP, BL, S)

    # labels on host (fp32): one-hot*SMOOTH | sim normalized
    se_ = np.asarray(inp["slot_emb"][b0:b0 + BL], np.float32).reshape(NSP, D)
    tgt = np.asarray(inp["tgt_slot_embs"], np.float32)

    def mlp32(x, w1, bb1, w2, bb2):
        return _mlp_np(x, np.asarray(w1, np.float32),
                       np.asarray(bb1, np.float32),
                       np.asarray(w2, np.float32),
                       np.asarray(bb2, np.float32))

    s_cat = np.concatenate([
        mlp32(se_, inp["Wps1"], inp["bps1"], inp["Wps2"], inp["bps2"]),
        mlp32(se_, inp["Wpc1"], inp["bpc1"], inp["Wpc2"], inp["bpc2"])],
        axis=-1)
    t_cat = np.concatenate([
        mlp32(tgt, inp["Wps1"], inp["bps1"], inp["Wps2"], inp["bps2"]),
        mlp32(tgt, inp["Wpc1"], inp["bpc1"], inp["Wpc2"], inp["bpc2"])],
        axis=-1)
    sn = np.maximum(np.linalg.norm(s_cat, axis=-1), EPS)
    tn = np.maximum(np.linalg.norm(t_cat, axis=-1), EPS)
    sim = (s_cat @ t_cat.T) / (sn[:, None] * tn[None, :])
    labsim = (sim / sim.sum(axis=-1, keepdims=True) * (1.0 - SMOOTH))
    sid = np.asarray(inp["src_slot_ids"][b0:b0 + BL], np.int64).reshape(NSP)
    oh = np.zeros((NSP, NS), np.float32)
    oh[np.arange(NSP), sid] = SMOOTH
    labs = np.concatenate([oh, labsim.astype(np.float32)], axis=1)

    def bf(a):
        return np.ascontiguousarray(np.asarray(a).astype(ml_dtypes.bfloat16))

    return {
        "hid": hid_bf, "gx": gx, "gh": gh,
        "wih": bf(wih), "whh": bf(whh),
        "cmask": bf(cmask), "labs": labs.astype(np.float32),
        "idb": bf(np.eye(128)), "bias2": bf(bias2), "ind2": bf(ind2),
    }, has_bias


_NC_CACHE = {}


def _get_nc(has_bias=False):
    if has_bias not in _NC_CACHE:
        _NC_CACHE[has_bias] = build_program(has_bias=has_bias)
    return _NC_CACHE[has_bias]


def kernel(**inputs):
    preps = [prep_core_inputs(inputs, ci) for ci in range(NCORES)]
    has_bias = any(p[1] for p in preps)
    in_maps = [p[0] for p in preps]
    nc = _get_nc(has_bias)
    res = bass_utils.run_bass_kernel_spmd(nc, in_maps, list(range(NCORES)))
    outs = [res.results[i]["out"].reshape(BL, K, OUTW) for i in range(NCORES)]
    return np.concatenate(outs, axis=0)
